# revision 1
# baseline (speedup 1.0000x reference)
"""Trainium2 Bass kernel for nn_EBlock (GNN message passing).

Strategy (8 NeuronCores, SPMD single program):
  * Edges are partitioned by DESTINATION node range (host-side sort), so the
    scatter-sum stays core-local -- no AllReduce of [N, HID] partials.
  * Node projection is shard-computed and AllGathered as a bf16 gather table.
  * Per-edge gather hv[src] is done with dma_gather (int16 indices), with the
    table split in two halves (lo/hi) because indices are int16.
  * The segment sum uses the sorted one-hot matmul trick: per 128-dst "bin",
    S[e, w] = (slot[e] == w) and h_bin += msgs_tile^T @ S_tile accumulated in
    PSUM on the TensorEngine.
  * LayerNorm statistics for the edge projection y = x @ W are computed via
    mu = x @ w_mean and E[y^2] = x^T (W W^T / HID) x (one small extra matmul
    plus batched vector ops), avoiding per-tile PSUM-bound reductions.
  * rstd = exp(-0.5 * ln(var + eps)) so the edge phase only ever needs the
    ln/exp activation table set (no per-block table switching).
"""

import os
import sys

sys.path.insert(0, "/opt/trn_rl_repo")

import numpy as np
import ml_dtypes

import concourse.bass as bass
import concourse.bacc as bacc
import concourse.mybir as mybir
import concourse.tile as tile
from concourse.tile import add_dep_helper
from concourse.bass_utils import run_bass_kernel_spmd

F16 = np.float16

DEBUG_NO_GATHER = False
DEBUG_NO_DEP = False

# ---------------------------------------------------------------- config

class Cfg:
    def __init__(self, n_nodes=50000, n_edges=800000, node_in=256, edge_in=64,
                 hid=128, out=16, n_cores=8, lo=32768, eps=1e-5):
        self.N, self.E = n_nodes, n_edges
        self.NODE_IN, self.EDGE_IN, self.HID, self.OUT = node_in, edge_in, hid, out
        self.NC = n_cores
        self.EPS = eps
        self.NPC = (n_nodes + n_cores - 1) // n_cores        # nodes per core
        self.NB = (self.NPC + 127) // 128                     # dst bins per core
        self.NPAD = self.NB * 128                             # padded shard rows
        self.AGROWS = self.NC * self.NPAD                     # allgather table rows
        self.LO = min(lo, self.AGROWS)                        # lo table rows
        self.HIR = self.AGROWS - self.LO                      # hi table rows
        assert self.LO <= 32768 and self.HIR <= 32768
        # K_LO / K_HI / TPB / ETOT set by prep()
        self.K_LO = self.K_HI = self.TPB = self.ETOT = None

    def key(self):
        return (self.N, self.E, self.NODE_IN, self.EDGE_IN, self.HID, self.OUT,
                self.NC, self.LO, self.K_LO, self.K_HI)


# ---------------------------------------------------------------- host prep

def _to_f16(x):
    return np.asarray(x, dtype=np.float32).astype(F16)


def prep(cfg, node_feats, edge_feats, src, dst,
         W_node, g_node, b_node, W_edge, g_edge, b_edge, W_out, g_out, b_out):
    """Shard/sort/pad the inputs.  Returns (in_maps, meta)."""
    N, E, NC = cfg.N, cfg.E, cfg.NC
    NPC, NB = cfg.NPC, cfg.NB
    HID, EIN, NIN, OUT = cfg.HID, cfg.EDGE_IN, cfg.NODE_IN, cfg.OUT

    src = np.asarray(src).astype(np.int64)
    dst = np.asarray(dst).astype(np.int64)
    node_feats = np.asarray(node_feats, dtype=np.float32)
    edge_feats = np.asarray(edge_feats, dtype=np.float32)

    # position of node i's hv row in the allgathered table
    src_remap = (src // NPC) * cfg.NPAD + (src % NPC)
    is_lo = src_remap < cfg.LO
    core_of_edge = dst // NPC

    percore = []
    for c in range(NC):
        sel = np.nonzero(core_of_edge == c)[0]
        d_loc = (dst[sel] - c * NPC).astype(np.int64)
        lo_cnt = np.bincount(d_loc[is_lo[sel]], minlength=NPC)
        hi_cnt = np.bincount(d_loc[~is_lo[sel]], minlength=NPC)

        # --- bin packing: NB bins of <=128 dst, balancing lo & hi loads
        order = np.argsort(-(lo_cnt + hi_cnt), kind="stable")
        bin_lo = np.zeros(NB); bin_hi = np.zeros(NB)
        bin_n = np.zeros(NB, np.int64)
        assign = np.full(NPC, -1, np.int64)
        slot = np.full(NPC, -1, np.int64)
        t_lo = max(lo_cnt.sum() / NB, 1.0)
        t_hi = max(hi_cnt.sum() / NB, 1.0)
        for d in order:
            cost = np.maximum((bin_lo + lo_cnt[d]) / t_lo,
                              (bin_hi + hi_cnt[d]) / t_hi)
            cost[bin_n >= 128] = np.inf
            b = int(np.argmin(cost))
            assign[d] = b
            slot[d] = bin_n[b]
            bin_n[b] += 1
            bin_lo[b] += lo_cnt[d]
            bin_hi[b] += hi_cnt[d]
        percore.append((sel, d_loc, assign, slot))

    # global tile counts (shared SPMD schedule)
    k_lo = k_hi = 1
    for c in range(NC):
        sel, d_loc, assign, slot = percore[c]
        lo_e = is_lo[sel]
        bin_of_edge = assign[d_loc]
        blc = np.bincount(bin_of_edge[lo_e], minlength=NB)
        bhc = np.bincount(bin_of_edge[~lo_e], minlength=NB)
        k_lo = max(k_lo, int(np.max((blc + 127) // 128)) if blc.size else 1)
        k_hi = max(k_hi, int(np.max((bhc + 127) // 128)) if bhc.size else 1)
    cfg.K_LO, cfg.K_HI = k_lo, k_hi
    cfg.TPB = k_lo + k_hi
    cfg.ETOT = NB * cfg.TPB * 128
    TPB, ETOT = cfg.TPB, cfg.ETOT

    # --- uniformity of gains/biases
    def uni(v):
        v = np.asarray(v, np.float32)
        return (float(v.flat[0]), True) if np.all(v == v.flat[0]) else (0.0, False)
    g_nu, node_g_uni = uni(g_node); b_nu, node_b_uni = uni(b_node)
    g_eu, edge_g_uni = uni(g_edge); b_eu, edge_b_uni = uni(b_edge)
    g_ou, out_g_uni = uni(g_out);  b_ou, out_b_uni = uni(b_out)

    meta = dict(g_nu=g_nu, b_nu=b_nu, g_eu=g_eu, b_eu=b_eu, g_ou=g_ou, b_ou=b_ou,
                node_uni=node_g_uni and node_b_uni,
                edge_uni=edge_g_uni and edge_b_uni,
                out_uni=out_g_uni and out_b_uni,
                inv=[])

    # --- shared weight arrays
    W_node = np.asarray(W_node, np.float32)
    W_edge = np.asarray(W_edge, np.float32)
    W_out = np.asarray(W_out, np.float32)
    assert NIN % 128 == 0
    KN = NIN // 128
    w_node_arr = np.ascontiguousarray(
        W_node.reshape(KN, 128, HID).transpose(1, 0, 2).reshape(128, KN * HID)
    ).astype(F16)
    w_edge_arr = _to_f16(W_edge)
    a_ext = np.zeros((EIN, EIN + 1), np.float64)
    a_ext[:, :EIN] = (W_edge.astype(np.float64) @ W_edge.astype(np.float64).T) / HID
    a_ext[:, EIN] = W_edge.astype(np.float64).mean(axis=1)
    a_ext_arr = a_ext.astype(np.float32).astype(F16)
    w_out_arr = np.asarray(W_out, np.float32)
    iota_arr = np.broadcast_to(
        np.tile(np.arange(128, dtype=np.float32), TPB)[None, :], (128, TPB * 128)
    ).astype(F16)
    g_edge_rep = np.broadcast_to(np.asarray(g_edge, np.float32)[None, :], (128, HID)).copy()
    b_edge_rep = np.broadcast_to(np.asarray(b_edge, np.float32)[None, :], (128, HID)).copy()
    g_node_rep = np.broadcast_to(np.asarray(g_node, np.float32)[None, :], (128, HID)).copy()
    b_node_rep = np.broadcast_to(np.asarray(b_node, np.float32)[None, :], (128, HID)).copy()
    g_out_rep = np.broadcast_to(np.asarray(g_out, np.float32)[None, :], (128, OUT)).copy()
    b_out_rep = np.broadcast_to(np.asarray(b_out, np.float32)[None, :], (128, OUT)).copy()

    in_maps = []
    for c in range(NC):
        sel, d_loc, assign, slot = percore[c]
        lo_e = is_lo[sel]
        bin_of_edge = assign[d_loc]
        slot_of_edge = slot[d_loc]

        # position of each real edge in the padded per-core stream
        ord_e = np.lexsort((src_remap[sel], (~lo_e).astype(np.int64), bin_of_edge))
        sel_o = sel[ord_e]
        bins_o = bin_of_edge[ord_e]
        lo_o = lo_e[ord_e]
        slot_o = slot_of_edge[ord_e]
        # rank within (bin, lo/hi) group
        grp = bins_o * 2 + (~lo_o).astype(np.int64)
        # edges are sorted by grp; rank = index - first index of grp
        first = np.zeros(2 * NB, np.int64)
        cnts = np.bincount(grp, minlength=2 * NB)
        np.cumsum(cnts[:-1], out=first[1:])
        rank = np.arange(len(grp)) - first[grp]
        base = bins_o * (TPB * 128) + np.where(lo_o, 0, k_lo * 128)
        pos = base + rank
        assert len(np.unique(pos)) == len(pos)

        ef_pad = np.zeros((ETOT, EIN), np.float32)
        ef_pad[pos] = edge_feats[sel_o]
        idx_pad = np.zeros(ETOT, np.int64)
        idx_pad[pos] = np.where(lo_o, src_remap[sel_o], src_remap[sel_o] - cfg.LO)
        slot_pad = np.full(ETOT, -1.0, np.float32)
        slot_pad[pos] = slot_o.astype(np.float32)

        edge_T = np.ascontiguousarray(ef_pad.T).astype(F16)
        x_rows = np.ascontiguousarray(
            ef_pad.reshape(NB * TPB, 128, EIN).transpose(1, 0, 2).reshape(128, NB * TPB * EIN)
        ).astype(F16)
        idx16 = idx_pad.astype(np.int16).reshape(ETOT // 16, 16).T  # [16, ETOT/16]
        src_w = np.ascontiguousarray(np.tile(idx16, (8, 1)))
        dst_sl = np.ascontiguousarray(
            slot_pad.reshape(NB * TPB, 128).T
        ).astype(F16)

        nshard = np.zeros((cfg.NPAD, NIN), np.float32)
        hi = min((c + 1) * NPC, N)
        nshard[: hi - c * NPC] = node_feats[c * NPC: hi]
        node_T = np.ascontiguousarray(nshard.T).astype(F16)

        in_maps.append({
            "edge_T": edge_T, "x_rows": x_rows, "src_w": src_w, "dst_sl": dst_sl,
            "node_T": node_T, "w_node": w_node_arr, "w_edge": w_edge_arr,
            "a_ext": a_ext_arr, "w_out": w_out_arr, "iota_in": iota_arr,
            "g_edge_rep": g_edge_rep, "b_edge_rep": b_edge_rep,
            "g_node_rep": g_node_rep, "b_node_rep": b_node_rep,
            "g_out_rep": g_out_rep, "b_out_rep": b_out_rep,
        })

        # output row of local dst d = assign[d]*128 + slot[d]
        real = np.arange(min(NPC, N - c * NPC))
        meta["inv"].append(assign[real] * 128 + slot[real])

    return in_maps, meta


# ---------------------------------------------------------------- device program

def build(cfg, meta):
    NB, TPB, K_LO, K_HI = cfg.NB, cfg.TPB, cfg.K_LO, cfg.K_HI
    HID, EIN, NIN, OUT = cfg.HID, cfg.EDGE_IN, cfg.NODE_IN, cfg.OUT
    ETOT, NPAD, AGROWS, LO = cfg.ETOT, cfg.NPAD, cfg.AGROWS, cfg.LO
    KN = NIN // 128
    EPS = cfg.EPS
    dt = mybir.dt
    f32, f16, i16 = dt.float32, dt.float16, dt.int16
    AX = mybir.AxisListType
    OP = mybir.AluOpType
    AF = mybir.ActivationFunctionType

    nc = bacc.Bacc("TRN2", target_bir_lowering=False, debug=False,
                   num_devices=cfg.NC)

    # register EPS as a usable constant bias AP for nc.scalar.activation
    _t = nc.alloc_sbuf_tensor(f"const-f32-eps", [128, 1], f32)
    nc.gpsimd.memset(_t.ap(), EPS)
    nc.const_aps.aps[(f32, EPS)] = _t.ap()
    nc.all_engine_barrier()

    def din(name, shape, d):
        return nc.dram_tensor(name, shape, d, kind="ExternalInput").ap()

    edge_T = din("edge_T", [EIN, ETOT], f16)
    x_rows = din("x_rows", [128, NB * TPB * EIN], f16)
    src_w = din("src_w", [128, ETOT // 16], i16)
    dst_sl = din("dst_sl", [128, NB * TPB], f16)
    node_T = din("node_T", [NIN, NPAD], f16)
    w_node = din("w_node", [128, KN * HID], f16)
    w_edge = din("w_edge", [EIN, HID], f16)
    a_ext = din("a_ext", [EIN, EIN + 1], f16)
    w_out = din("w_out", [HID, OUT], f32)
    iota_in = din("iota_in", [128, TPB * 128], f16)
    g_edge_rep = din("g_edge_rep", [128, HID], f32)
    b_edge_rep = din("b_edge_rep", [128, HID], f32)
    g_node_rep = din("g_node_rep", [128, HID], f32)
    b_node_rep = din("b_node_rep", [128, HID], f32)
    g_out_rep = din("g_out_rep", [128, OUT], f32)
    b_out_rep = din("b_out_rep", [128, OUT], f32)
    out_ext = nc.dram_tensor("out", [NB * 128, OUT], f32, kind="ExternalOutput").ap()

    hv_in = nc.dram_tensor("hv_in", [NPAD, HID], f16).ap()
    hv_ag = nc.dram_tensor("hv_ag", [AGROWS, HID], f16, addr_space="Shared").ap()
    hv_loc = nc.dram_tensor("hv_loc", [AGROWS, HID], f16).ap()

    g_nu, b_nu = meta["g_nu"], meta["b_nu"]
    g_eu, b_eu = meta["g_eu"], meta["b_eu"]
    g_ou, b_ou = meta["g_ou"], meta["b_ou"]

    with tile.TileContext(nc) as tc:
        cpool = tc.alloc_tile_pool(name="consts", bufs=1)
        ppool = tc.alloc_tile_pool(name="persist", bufs=1)
        spool = tc.alloc_tile_pool(name="stats", bufs=2)
        wkpool = tc.alloc_tile_pool(name="work", bufs=2)
        gpool = tc.alloc_tile_pool(name="gath", bufs=2)
        pspool = tc.alloc_tile_pool(name="ps", bufs=3, space="PSUM")
        t1pool = tc.alloc_tile_pool(name="t1", bufs=2, space="PSUM")
        hbpool = tc.alloc_tile_pool(name="hb", bufs=2, space="PSUM")

        # ---- constants into SBUF
        wnode_sb = cpool.tile([128, KN, HID], f16)
        nc.sync.dma_start(out=wnode_sb[:], in_=w_node[:])
        wedge_sb = cpool.tile([EIN, HID], f16)
        nc.sync.dma_start(out=wedge_sb[:], in_=w_edge[:])
        aext_sb = cpool.tile([EIN, EIN + 1], f16)
        nc.sync.dma_start(out=aext_sb[:], in_=a_ext[:])
        wout_sb = cpool.tile([HID, OUT], f32)
        nc.sync.dma_start(out=wout_sb[:], in_=w_out[:])
        iota_sb = cpool.tile([128, TPB, 128], f16)
        nc.sync.dma_start(out=iota_sb[:], in_=iota_in[:])
        srcw_sb = cpool.tile([128, ETOT // 16], i16)
        nc.sync.dma_start(out=srcw_sb[:], in_=src_w[:])
        dst_sb = cpool.tile([128, NB * TPB], f16)
        nc.sync.dma_start(out=dst_sb[:], in_=dst_sl[:])
        if not meta["edge_uni"]:
            ger_sb = cpool.tile([128, HID], f32)
            nc.sync.dma_start(out=ger_sb[:], in_=g_edge_rep[:])
            ber_sb = cpool.tile([128, HID], f32)
            nc.sync.dma_start(out=ber_sb[:], in_=b_edge_rep[:])
        if not meta["node_uni"]:
            gnr_sb = cpool.tile([128, HID], f32)
            nc.sync.dma_start(out=gnr_sb[:], in_=g_node_rep[:])
            bnr_sb = cpool.tile([128, HID], f32)
            nc.sync.dma_start(out=bnr_sb[:], in_=b_node_rep[:])
        if not meta["out_uni"]:
            gor_sb = cpool.tile([128, OUT], f32)
            nc.sync.dma_start(out=gor_sb[:], in_=g_out_rep[:])
            bor_sb = cpool.tile([128, OUT], f32)
            nc.sync.dma_start(out=bor_sb[:], in_=b_out_rep[:])

        # =================================================== phase N: hv
        g_all = ppool.tile([128, NB * HID], f16, tag="g_all")
        ex2_n = spool.tile([128, NB], f32, tag="ex2n")
        sum_n = spool.tile([128, NB], f32, tag="sumn")
        node_r = node_T.rearrange("(a p) m -> p a m", p=128)
        sqj = None
        for t in range(NB):
            nt = wkpool.tile([128, KN, 128], f16, tag="nt")
            nc.sync.dma_start(out=nt[:], in_=node_r[:, :, t * 128:(t + 1) * 128])
            ps = pspool.tile([128, HID], f32, tag="mmout")
            for k in range(KN):
                nc.tensor.matmul(ps[:], lhsT=nt[:, k, :], rhs=wnode_sb[:, k, :],
                                 start=(k == 0), stop=(k == KN - 1))
            gsl = g_all[:, t * HID:(t + 1) * HID]
            nc.scalar.activation(out=gsl, in_=ps[:], func=AF.Gelu)
            sqj = wkpool.tile([128, HID], f16, tag="sqj")
            nc.vector.scalar_tensor_tensor(
                out=sqj[:], in0=gsl, scalar=1.0, in1=gsl,
                op0=OP.mult, op1=OP.mult, accum_out=ex2_n[:, t:t + 1])
            nc.vector.reduce_sum(out=sum_n[:, t:t + 1], in_=gsl, axis=AX.X)

        mu_n = spool.tile([128, NB], f32, tag="mun")
        nc.vector.tensor_scalar(out=mu_n[:], in0=sum_n[:], scalar1=1.0 / HID,
                                scalar2=None, op0=OP.mult)
        nc.vector.tensor_scalar(out=ex2_n[:], in0=ex2_n[:], scalar1=1.0 / HID,
                                scalar2=None, op0=OP.mult)
        tmp_n = spool.tile([128, NB], f32, tag="tmpn")
        nc.vector.scalar_tensor_tensor(out=tmp_n[:], in0=mu_n[:], scalar=-1.0,
                                       in1=mu_n[:], op0=OP.mult, op1=OP.mult)
        var_n = spool.tile([128, NB], f32, tag="varn")
        nc.vector.tensor_tensor(out=var_n[:], in0=tmp_n[:], in1=ex2_n[:], op=OP.add)
        lnv_n = spool.tile([128, NB], f32, tag="lnvn")
        nc.scalar.activation(out=lnv_n[:], in_=var_n[:], func=AF.Ln, bias=EPS)
        rstd_n = spool.tile([128, NB], f32, tag="rstdn")
        nc.scalar.activation(out=rstd_n[:], in_=lnv_n[:], func=AF.Exp, scale=-0.5)
        if meta["node_uni"]:
            rs2_n = spool.tile([128, NB], f32, tag="rs2n")
            nc.vector.tensor_scalar(out=rs2_n[:], in0=rstd_n[:], scalar1=g_nu,
                                    scalar2=None, op0=OP.mult)
            nb_n = spool.tile([128, NB], f32, tag="nbn")
            nc.vector.scalar_tensor_tensor(out=nb_n[:], in0=mu_n[:], scalar=-1.0,
                                           in1=rs2_n[:], op0=OP.mult, op1=OP.mult)
            if b_nu != 0.0:
                nc.vector.tensor_scalar(out=nb_n[:], in0=nb_n[:], scalar1=b_nu,
                                        scalar2=None, op0=OP.add)
        for t in range(NB):
            hv_t = wkpool.tile([128, HID], f16, tag="hvt")
            gsl = g_all[:, t * HID:(t + 1) * HID]
            if meta["node_uni"]:
                nc.vector.tensor_scalar(out=hv_t[:], in0=gsl,
                                        scalar1=rs2_n[:, t:t + 1],
                                        scalar2=nb_n[:, t:t + 1],
                                        op0=OP.mult, op1=OP.add)
            else:
                zt = wkpool.tile([128, HID], f32, tag="zt")
                nc.vector.tensor_scalar(out=zt[:], in0=gsl,
                                        scalar1=mu_n[:, t:t + 1],
                                        scalar2=rstd_n[:, t:t + 1],
                                        op0=OP.subtract, op1=OP.mult)
                nc.vector.tensor_tensor(out=zt[:], in0=zt[:], in1=gnr_sb[:], op=OP.mult)
                nc.vector.tensor_tensor(out=hv_t[:], in0=zt[:], in1=bnr_sb[:], op=OP.add)
            nc.sync.dma_start(out=hv_in[t * 128:(t + 1) * 128, :], in_=hv_t[:])

        nc.gpsimd.collective_compute(
            "AllGather", OP.bypass,
            replica_groups=[list(range(cfg.NC))],
            ins=[hv_in[:]], outs=[hv_ag[:]],
        )
        nc.sync.dma_start(out=hv_loc[:], in_=hv_ag[:])

        # =================================================== phase E: edges
        h_sb = ppool.tile([128, NB * 128], f32, tag="h_sb")
        GRP = 7
        last_exp = None
        for b in range(NB):
            eT = wkpool.tile([EIN, TPB * 128], f16, tag="eT")
            nc.sync.dma_start(out=eT[:], in_=edge_T[:, b * TPB * 128:(b + 1) * TPB * 128])
            xr = wkpool.tile([128, TPB, EIN], f16, tag="xr")
            nc.sync.dma_start(
                out=xr[:], in_=x_rows[:, b * TPB * EIN:(b + 1) * TPB * EIN])
            gb = gpool.tile([128, TPB, HID], f16, tag="gb")
            col0 = b * TPB * 8
            # dma_gather calls capped at 512 indices (4 tiles): larger
            # num_idxs crashes the device (HW packet limit).
            def emit_gathers(t_base, ntiles, src_view):
                done = 0
                while done < ntiles:
                    step = min(4, ntiles - done)
                    nidx = step * 128
                    nc.gpsimd.dma_gather(
                        out_ap=gb[:, t_base + done:t_base + done + step, :],
                        in_ap=src_view,
                        idxs_ap=srcw_sb[:, col0 + (t_base + done) * 8:
                                        col0 + (t_base + done + step) * 8],
                        num_idxs=nidx, num_idxs_reg=nidx, elem_size=HID)
                    done += step
            if K_LO > 0:
                emit_gathers(0, K_LO, hv_loc[0:LO, :])
            if K_HI > 0:
                emit_gathers(K_LO, K_HI, hv_loc[LO:AGROWS, :])

            mu_e = spool.tile([128, TPB], f32, tag="mue")
            q_e = spool.tile([128, TPB], f32, tag="qe")
            for g0 in range(0, TPB, GRP):
                gl = min(GRP, TPB - g0)
                t1g = t1pool.tile([128, GRP, EIN + 1], f32, tag="t1g")
                for j in range(gl):
                    tt = g0 + j
                    nc.tensor.matmul(t1g[:, j, :],
                                     lhsT=eT[:, tt * 128:(tt + 1) * 128],
                                     rhs=aext_sb[:], start=True, stop=True)
                p2 = wkpool.tile([128, GRP, EIN], f16, tag="p2")
                nc.vector.tensor_tensor(out=p2[:, :gl, :], in0=t1g[:, :gl, :EIN],
                                        in1=xr[:, g0:g0 + gl, :], op=OP.mult)
                nc.vector.reduce_sum(out=q_e[:, g0:g0 + gl], in_=p2[:, :gl, :],
                                     axis=AX.X)
                nc.vector.tensor_copy(out=mu_e[:, g0:g0 + gl], in_=t1g[:, :gl, EIN])

            tmp_e = spool.tile([128, TPB], f32, tag="tmpe")
            nc.vector.scalar_tensor_tensor(out=tmp_e[:], in0=mu_e[:], scalar=-1.0,
                                           in1=mu_e[:], op0=OP.mult, op1=OP.mult)
            var_e = spool.tile([128, TPB], f32, tag="vare")
            nc.vector.tensor_tensor(out=var_e[:], in0=tmp_e[:], in1=q_e[:], op=OP.add)
            lnv_e = spool.tile([128, TPB], f32, tag="lnve")
            nc.scalar.activation(out=lnv_e[:], in_=var_e[:], func=AF.Ln, bias=EPS)
            rstd_e = spool.tile([128, TPB], f32, tag="rstde")
            nc.scalar.activation(out=rstd_e[:], in_=lnv_e[:], func=AF.Exp, scale=-0.5)
            if meta["edge_uni"]:
                rs2_e = spool.tile([128, TPB], f32, tag="rs2e")
                nc.vector.tensor_scalar(out=rs2_e[:], in0=rstd_e[:], scalar1=g_eu,
                                        scalar2=None, op0=OP.mult)
                nb_e = spool.tile([128, TPB], f32, tag="nbe")
                nc.vector.scalar_tensor_tensor(out=nb_e[:], in0=mu_e[:], scalar=-1.0,
                                               in1=rs2_e[:], op0=OP.mult, op1=OP.mult)
                if b_eu != 0.0:
                    nc.vector.tensor_scalar(out=nb_e[:], in0=nb_e[:], scalar1=b_eu,
                                            scalar2=None, op0=OP.add)

            he = wkpool.tile([128, TPB, HID], f16, tag="he")
            for t in range(TPB):
                hp = pspool.tile([128, HID], f32, tag="mmout")
                nc.tensor.matmul(hp[:], lhsT=eT[:, t * 128:(t + 1) * 128],
                                 rhs=wedge_sb[:], start=True, stop=True)
                if meta["edge_uni"]:
                    last_exp = nc.scalar.activation(
                        out=he[:, t, :], in_=hp[:], func=AF.Exp,
                        scale=rs2_e[:, t:t + 1], bias=nb_e[:, t:t + 1])
                else:
                    zt = wkpool.tile([128, HID], f32, tag="zte")
                    nc.vector.tensor_scalar(out=zt[:], in0=hp[:],
                                            scalar1=mu_e[:, t:t + 1],
                                            scalar2=rstd_e[:, t:t + 1],
                                            op0=OP.subtract, op1=OP.mult)
                    nc.vector.tensor_tensor(out=zt[:], in0=zt[:], in1=ger_sb[:],
                                            op=OP.mult)
                    nc.vector.tensor_tensor(out=zt[:], in0=zt[:], in1=ber_sb[:],
                                            op=OP.add)
                    last_exp = nc.scalar.activation(out=he[:, t, :], in_=zt[:],
                                                    func=AF.Exp)

            msgs = wkpool.tile([128, TPB, HID], f16, tag="msgs")
            nc.vector.tensor_tensor(out=msgs[:], in0=he[:], in1=gb[:], op=OP.mult)
            S = wkpool.tile([128, TPB, 128], f16, tag="S")
            dsl = dst_sb[:, b * TPB:(b + 1) * TPB, None].to_broadcast([128, TPB, 128])
            nc.vector.tensor_tensor(out=S[:], in0=iota_sb[:], in1=dsl, op=OP.is_equal)

            hb = hbpool.tile([128, 128], f32, tag="hb")
            for t in range(TPB):
                nc.tensor.matmul(hb[:], lhsT=msgs[:, t, :], rhs=S[:, t, :],
                                 start=(t == 0), stop=(t == TPB - 1))
            nc.vector.tensor_copy(out=h_sb[:, b * 128:(b + 1) * 128], in_=hb[:])

        # =================================================== phase OUT
        go_all = ppool.tile([128, NB * OUT], f32, tag="go_all")
        ex2_o = spool.tile([128, NB], f32, tag="ex2o")
        sum_o = spool.tile([128, NB], f32, tag="sumo")
        for b in range(NB):
            op_ps = pspool.tile([128, HID], f32, tag="mmout")   # shares "ps" pool; use [:, :OUT]
            nc.tensor.matmul(op_ps[:, :OUT], lhsT=h_sb[:, b * 128:(b + 1) * 128],
                             rhs=wout_sb[:], start=True, stop=True)
            osl = go_all[:, b * OUT:(b + 1) * OUT]
            gelu_i = nc.scalar.activation(out=osl, in_=op_ps[:, :OUT], func=AF.Gelu)
            if last_exp is not None and not DEBUG_NO_DEP:
                add_dep_helper(gelu_i.ins, last_exp.ins, sync=False,
                               reason="keep OUT-phase gelu after edge-phase exp (ACT tables)")
            sqo = wkpool.tile([128, OUT], f16, tag="sqo")
            nc.vector.scalar_tensor_tensor(
                out=sqo[:], in0=osl, scalar=1.0, in1=osl,
                op0=OP.mult, op1=OP.mult, accum_out=ex2_o[:, b:b + 1])
            nc.vector.reduce_sum(out=sum_o[:, b:b + 1], in_=osl, axis=AX.X)

        mu_o = spool.tile([128, NB], f32, tag="muo")
        nc.vector.tensor_scalar(out=mu_o[:], in0=sum_o[:], scalar1=1.0 / OUT,
                                scalar2=None, op0=OP.mult)
        nc.vector.tensor_scalar(out=ex2_o[:], in0=ex2_o[:], scalar1=1.0 / OUT,
                                scalar2=None, op0=OP.mult)
        tmp_o = spool.tile([128, NB], f32, tag="tmpo")
        nc.vector.scalar_tensor_tensor(out=tmp_o[:], in0=mu_o[:], scalar=-1.0,
                                       in1=mu_o[:], op0=OP.mult, op1=OP.mult)
        var_o = spool.tile([128, NB], f32, tag="varo")
        nc.vector.tensor_tensor(out=var_o[:], in0=tmp_o[:], in1=ex2_o[:], op=OP.add)
        lnv_o = spool.tile([128, NB], f32, tag="lnvo")
        nc.scalar.activation(out=lnv_o[:], in_=var_o[:], func=AF.Ln, bias=EPS)
        rstd_o = spool.tile([128, NB], f32, tag="rstdo")
        nc.scalar.activation(out=rstd_o[:], in_=lnv_o[:], func=AF.Exp, scale=-0.5)
        if meta["out_uni"]:
            rs2_o = spool.tile([128, NB], f32, tag="rs2o")
            nc.vector.tensor_scalar(out=rs2_o[:], in0=rstd_o[:], scalar1=g_ou,
                                    scalar2=None, op0=OP.mult)
            nb_o = spool.tile([128, NB], f32, tag="nbo")
            nc.vector.scalar_tensor_tensor(out=nb_o[:], in0=mu_o[:], scalar=-1.0,
                                           in1=rs2_o[:], op0=OP.mult, op1=OP.mult)
            if b_ou != 0.0:
                nc.vector.tensor_scalar(out=nb_o[:], in0=nb_o[:], scalar1=b_ou,
                                        scalar2=None, op0=OP.add)
        out_all = ppool.tile([128, NB, OUT], f32, tag="out_all")
        for b in range(NB):
            osl = go_all[:, b * OUT:(b + 1) * OUT]
            if meta["out_uni"]:
                nc.vector.tensor_scalar(out=out_all[:, b, :], in0=osl,
                                        scalar1=rs2_o[:, b:b + 1],
                                        scalar2=nb_o[:, b:b + 1],
                                        op0=OP.mult, op1=OP.add)
            else:
                zo = wkpool.tile([128, OUT], f32, tag="zo")
                nc.vector.tensor_scalar(out=zo[:], in0=osl,
                                        scalar1=mu_o[:, b:b + 1],
                                        scalar2=rstd_o[:, b:b + 1],
                                        op0=OP.subtract, op1=OP.mult)
                nc.vector.tensor_tensor(out=zo[:], in0=zo[:], in1=gor_sb[:], op=OP.mult)
                nc.vector.tensor_tensor(out=out_all[:, b, :], in0=zo[:], in1=bor_sb[:],
                                        op=OP.add)
        out_r = out_ext.rearrange("(b p) o -> p b o", p=128)
        nc.sync.dma_start(out=out_r[:], in_=out_all[:])

        for p in (hbpool, t1pool, pspool, gpool, wkpool, spool, ppool, cpool):
            p.release()

    nc.compile()
    return nc


# ---------------------------------------------------------------- entry point

_CACHE = {}


def _get_program(cfg, meta):
    key = cfg.key() + (meta["node_uni"], meta["edge_uni"], meta["out_uni"],
                       meta["g_nu"], meta["b_nu"], meta["g_eu"], meta["b_eu"],
                       meta["g_ou"], meta["b_ou"])
    if key not in _CACHE:
        _CACHE[key] = build(cfg, meta)
    return _CACHE[key]


def run(cfg, inputs, trace=False, trace_cores=None):
    in_maps, meta = prep(cfg, **inputs)
    nc = _get_program(cfg, meta)
    res = run_bass_kernel_spmd(nc, in_maps, core_ids=list(range(cfg.NC)),
                               trace=trace, trace_cores=trace_cores)
    out = np.empty((cfg.N, cfg.OUT), np.float32)
    for c in range(cfg.NC):
        oc = res.results[c]["out"]
        lo = c * cfg.NPC
        hi = min((c + 1) * cfg.NPC, cfg.N)
        out[lo:hi] = oc[meta["inv"][c]]
    return out, res


def kernel(node_feats, edge_feats, src, dst,
           W_node, g_node, b_node, W_edge, g_edge, b_edge,
           W_out, g_out, b_out):
    cfg = Cfg(n_nodes=node_feats.shape[0], n_edges=edge_feats.shape[0],
              node_in=node_feats.shape[1], edge_in=edge_feats.shape[1],
              hid=W_node.shape[1], out=W_out.shape[1])
    out, _ = run(cfg, dict(
        node_feats=node_feats, edge_feats=edge_feats, src=src, dst=dst,
        W_node=W_node, g_node=g_node, b_node=b_node,
        W_edge=W_edge, g_edge=g_edge, b_edge=b_edge,
        W_out=W_out, g_out=g_out, b_out=b_out))
    return out

